# revision 34
# baseline (speedup 1.0000x reference)
"""Trainium2 Bass kernel for 8-head MultiHeadAttention (B=2, S=4096, E=512).

Sharding: 8 cores = 2 batches x 4 query-row chunks of 1024; each core computes
all 8 heads for its (batch, q-range), so there are no collectives.

Structure (v3):
- scores = (Wq xq)^T (Wk xk) = (M^T xq)^T xk with M = Wq^T Wk folded on host:
  the key side needs NO on-device projection (raw key columns are the
  stationary operand), and only the small q-side projection runs on the PE.
- QK^T and attn@V run in bf16 (fp8 was tried and measured: the heavy-tailed
  softmax here has n_eff ~ 2..10, so fp8's quantization noise does not
  average out and blows the 2e-2 gate).
- exp runs on ACT for ~90% of tiles; ~10% of (kc,qw) slots are offloaded to
  the DVE via a Schraudolph bit-trick: int16(round(s*128*log2(e)/8 + 16248.67))
  IS the bf16 bit pattern of ~exp(s/8), computed by one scalar_tensor_tensor
  ((ps + 703.921) * m32) where m32 is {23.0831, 0} so the mask rides along and
  masked lanes become +0.0. The 16248.67 (vs 16256) bias zero-means the
  mantissa-interpolation sawtooth so softmax cancels it; measured L2 8e-3.
- masks for ACT tiles are multiplicative bf16 (DVE at 2x, some tiles on Pool).
- V is unprojected (+ ones column for the denominator); Wv is applied after
  normalization on the small [64, q] normalized context. The output bias bo
  is added by a K=1 ones-row matmul into the outproj PSUM.
"""
import sys
for _p in ('/root/.axon_site/_ro/trn_rl_repo', '/opt/trn_rl_repo'):
    if _p not in sys.path:
        sys.path.append(_p)

import numpy as np
import ml_dtypes

import concourse.bass as bass
import concourse.tile as tile
from concourse import bacc, mybir
from concourse import bass_utils

F32 = mybir.dt.float32
BF16 = mybir.dt.bfloat16
I16 = mybir.dt.int16
AF = mybir.ActivationFunctionType
ALU = mybir.AluOpType

N_CORES = 8
B, S, E, H, DH = 2, 4096, 512, 8, 64
QLEN = S // 4           # 1024 q rows per core
KC = S // 128           # 32 k chunks
VBLK = H * 65           # 520: per-kc block in valp

# Schraudolph (int16 -> bf16 bits): y16 = s*128*log2(e)/8 + (16256 - 7.33)
SCH_SCALE = 23.083120654223414        # 128*log2(e)/8
SCH_BIAS = 16248.67 / SCH_SCALE       # 703.9209...: (ps + bias)*m32

# (kc,qw) slot classes: s = 2*kc + qw
ASSIST_SLOTS = frozenset({18, 26, 42, 58, 17, 33, 49, 21})      # DVE exp (8)
POOL_SLOTS = frozenset(s for s in range(64)
                       if s % 6 == 2 and s not in ASSIST_SLOTS)  # Pool mask

_CACHE = {}


def _build_module():
    nc = bacc.Bacc("TRN2", target_bir_lowering=False, debug=False,
                   enable_asserts=True, num_devices=N_CORES)

    xqT = nc.dram_tensor("xqT", [E, QLEN], BF16, kind="ExternalInput").ap()
    xkT = nc.dram_tensor("xkT", [E, S], BF16, kind="ExternalInput").ap()
    valpd = nc.dram_tensor("valpd", [128, KC * VBLK], BF16, kind="ExternalInput").ap()
    m16d = nc.dram_tensor("m16d", [128, KC * QLEN], BF16, kind="ExternalInput").ap()
    m32d = nc.dram_tensor("m32d", [128, len(ASSIST_SLOTS) * 512], F32,
                          kind="ExternalInput").ap()
    wmd = nc.dram_tensor("wmd", [128, DH], BF16, kind="ExternalInput").ap()
    wvT = nc.dram_tensor("wvT", [DH, DH], BF16, kind="ExternalInput").ap()
    wod = nc.dram_tensor("wod", [128, 4 * E], BF16, kind="ExternalInput").ap()
    bo_bd = nc.dram_tensor("bo_bd", [128, E], F32, kind="ExternalInput").ap()
    out = nc.dram_tensor("out", [QLEN, E], F32, kind="ExternalOutput").ap()

    with tile.TileContext(nc) as tc:
        _emit(tc, nc, xqT, xkT, valpd, m16d, m32d, wmd, wvT, wod, bo_bd, out)

    nc.compile()
    return nc


def _emit(tc, nc, xqT, xkT, valpd, m16d, m32d, wmd, wvT, wod, bo_bd, out):
    from contextlib import ExitStack
    ctx = ExitStack()
    const = ctx.enter_context(tc.tile_pool(name="const", bufs=1))
    qpool = ctx.enter_context(tc.tile_pool(name="qp", bufs=1))
    xqpool = ctx.enter_context(tc.tile_pool(name="xq", bufs=1))
    ptpool = ctx.enter_context(tc.tile_pool(name="pt", bufs=3))
    usbp = ctx.enter_context(tc.tile_pool(name="usb", bufs=4))
    rcpool = ctx.enter_context(tc.tile_pool(name="rc", bufs=4))
    ospool = ctx.enter_context(tc.tile_pool(name="osb", bufs=2))
    psp = ctx.enter_context(tc.tile_pool(name="psp", bufs=3, space="PSUM"))
    uacc = ctx.enter_context(tc.tile_pool(name="uacc", bufs=2, space="PSUM"))

    # ---- small constants (critical path first) ----
    wm_sb = const.tile([128, DH], BF16, tag="wm")   # M at rows 0:64 and 64:128
    nc.sync.dma_start(wm_sb, wmd)
    ones_sb = const.tile([65, DH], BF16, tag="ones")
    nc.vector.memset(ones_sb, 1.0)

    xq_sb = []   # per pair: [128, QLEN] bf16, head-even rows 0:64, odd 64:128
    for pair in range(4):
        xq_sb.append(xqpool.tile([128, QLEN], BF16, tag=f"xq{pair}",
                                 name=f"xq{pair}"))

    def load_xq(pair):
        nc.sync.dma_start(xq_sb[pair], xqT[(2 * pair) * DH:(2 * pair + 2) * DH, :])

    load_xq(0)

    xkp = []     # per pair: [128, S] bf16, head-even rows 0:64, odd 64:128
    for pair in range(4):
        xkp.append(const.tile([128, S], BF16, tag=f"xk{pair}", name=f"xk{pair}"))

    def load_xk(pair):
        nc.sync.dma_start(xkp[pair], xkT[(2 * pair) * DH:(2 * pair + 2) * DH, :])

    nc.sync.dma_start(xkp[0][:, 0:1024], xkT[0:128, 0:1024])

    valp = const.tile([128, KC * VBLK], BF16, tag="valp")
    m16 = const.tile([128, KC * QLEN], BF16, tag="m16")
    m32 = const.tile([128, len(ASSIST_SLOTS) * 512], F32, tag="m32")
    wv_sb = const.tile([DH, DH], BF16, tag="wv")
    wo_sb = const.tile([128, 4 * E], BF16, tag="wo")
    bo_sb = const.tile([128, E], F32, tag="bo")

    def m16_chunk(qw, kclo, kchi):
        """Strided (qw-half, kc-range) slice of m16 / m16d for need-ordered DMA."""
        n = kchi - kclo
        dst = bass.AP(tensor=m16.tensor,
                      offset=m16.offset + kclo * QLEN + qw * 512,
                      ap=[m16.ap[0], [QLEN, n], [1, 512]])
        srcbase = m16d[:, 0:]
        srcd = bass.AP(tensor=srcbase.tensor,
                       offset=srcbase.offset + kclo * QLEN + qw * 512,
                       ap=[srcbase.ap[0], [QLEN, n], [1, 512]])
        return dst, srcd

    # Startup ladder: call 0 is DMA-paced, everything in strict need-order.
    nc.sync.dma_start(*m16_chunk(0, 0, 4))
    nc.sync.dma_start(valp[:, 0:4 * VBLK], valpd[:, 0:4 * VBLK])
    nc.gpsimd.dma_start(xq_sb[1], xqT[2 * DH:4 * DH, :])
    nc.gpsimd.dma_start(xq_sb[2], xqT[4 * DH:6 * DH, :])
    nc.gpsimd.dma_start(xq_sb[3], xqT[6 * DH:8 * DH, :])
    nc.gpsimd.dma_start(m32, m32d)

    def load_bulk():
        nc.sync.dma_start(*m16_chunk(0, 4, 10))
        nc.sync.dma_start(valp[:, 4 * VBLK:10 * VBLK], valpd[:, 4 * VBLK:10 * VBLK])
        nc.sync.dma_start(xkp[0][:, 1024:4096], xkT[0:128, 1024:4096])
        nc.sync.dma_start(*m16_chunk(0, 10, 18))
        nc.sync.dma_start(valp[:, 10 * VBLK:18 * VBLK], valpd[:, 10 * VBLK:18 * VBLK])
        nc.sync.dma_start(*m16_chunk(0, 18, 26))
        nc.sync.dma_start(valp[:, 18 * VBLK:26 * VBLK], valpd[:, 18 * VBLK:26 * VBLK])
        nc.sync.dma_start(*m16_chunk(0, 26, 32))
        nc.sync.dma_start(valp[:, 26 * VBLK:32 * VBLK], valpd[:, 26 * VBLK:32 * VBLK])
        load_xk(1)
        nc.sync.dma_start(*m16_chunk(1, 0, 16))
        nc.sync.dma_start(*m16_chunk(1, 16, 32))
        nc.gpsimd.dma_start(wv_sb, wvT)
        nc.gpsimd.dma_start(xkp[2], xkT[4 * DH:6 * DH, :])
        nc.gpsimd.dma_start(xkp[3], xkT[6 * DH:8 * DH, :])
        nc.gpsimd.dma_start(wo_sb, wod)
        nc.gpsimd.dma_start(bo_sb, bo_bd)

    qp = [qpool.tile([128, QLEN], BF16, tag=f"qp{pair}", name=f"qp{pair}")
          for pair in range(4)]

    # ---- q-side projection: qp[pair] = bf16(M^T xq) ----
    def qproj_unit(pair, qc):
        def go():
            ut = psp.tile([128, 1024], F32, tag="ps", name=f"qp{pair}_{qc}")
            for h2 in range(2):
                nc.tensor.matmul(
                    ut[h2 * 64:(h2 + 1) * 64, 0:512],
                    lhsT=wm_sb[h2 * 64:(h2 + 1) * 64, :],
                    rhs=xq_sb[pair][h2 * 64:(h2 + 1) * 64,
                                    qc * 512:(qc + 1) * 512],
                    start=True, stop=True)
            nc.vector.tensor_copy(qp[pair][:, qc * 512:(qc + 1) * 512],
                                  ut[:, 0:512])
        return go

    def m16_ap(kc, qw):
        t = m16[:, kc * QLEN + qw * 512:]
        return bass.AP(tensor=t.tensor, offset=t.offset,
                       ap=[t.ap[0], [0, 2], [1, 512]])

    def m32_ap(slot_idx):
        t = m32[:, slot_idx * 512:]
        return bass.AP(tensor=t.tensor, offset=t.offset,
                       ap=[t.ap[0], [0, 2], [1, 512]])

    _slot_order = sorted(ASSIST_SLOTS)

    concatT = [const.tile([128, QLEN], BF16, tag=f"ct{pair}", name=f"ct{pair}")
               for pair in range(4)]

    def attn(pair, qw, trickle=(), prelude=(), last=False):
        trickle = list(trickle)
        prelude = list(prelude)
        U = [uacc.tile([65, 512], F32, tag="u", name=f"U{pair}_{qw}_{h2}")
             for h2 in range(2)]

        def attnv(kc, pt, half):
            def go():
                for h2 in range(2):
                    nc.tensor.matmul(
                        U[h2],
                        lhsT=valp[:, kc * VBLK + (2 * pair + h2) * 65:
                                  kc * VBLK + (2 * pair + h2) * 65 + 65],
                        rhs=pt[:, half * 1024 + h2 * 512:
                               half * 1024 + (h2 + 1) * 512],
                        start=(kc == 0), stop=(kc == KC - 1),
                        skip_group_check=True)
            return go

        avq = []
        for j in range(KC // 2):
            pt = ptpool.tile([128, 2048], BF16, tag="pt",
                             name=f"pt{pair}_{qw}_{j}")
            pti16 = pt.bitcast(I16)
            for half in range(2):
                kc = 2 * j + half
                slot = 2 * kc + qw
                ps = psp.tile([128, 1024], F32, tag="ps",
                              name=f"ps{pair}_{qw}_{kc}")
                for h2 in range(2):
                    nc.tensor.matmul(
                        ps[:, h2 * 512:(h2 + 1) * 512],
                        lhsT=xkp[pair][h2 * 64:(h2 + 1) * 64,
                                       kc * 128:(kc + 1) * 128],
                        rhs=qp[pair][h2 * 64:(h2 + 1) * 64,
                                     qw * 512:(qw + 1) * 512],
                        start=True, stop=True)
                dst = pt[:, half * 1024:(half + 1) * 1024]
                if slot in ASSIST_SLOTS:
                    dsti = pti16[:, half * 1024:(half + 1) * 1024]
                    dv = bass.AP(tensor=dsti.tensor, offset=dsti.offset,
                                 ap=[dsti.ap[0], [512, 2], [1, 512]])
                    pv = bass.AP(tensor=ps.tensor, offset=ps.offset,
                                 ap=[ps.ap[0], [512, 2], [1, 512]])
                    nc.vector.scalar_tensor_tensor(
                        dv, pv, SCH_BIAS, m32_ap(_slot_order.index(slot)),
                        ALU.add, ALU.mult)
                else:
                    nc.scalar.activation(dst, ps, AF.Exp, bias=0.0, scale=0.125)
                    dv = bass.AP(tensor=dst.tensor, offset=dst.offset,
                                 ap=[dst.ap[0], [512, 2], [1, 512]])
                    eng = nc.gpsimd if slot in POOL_SLOTS else nc.vector
                    eng.tensor_mul(dv, dv, m16_ap(kc, qw))
            # the previous call's tail (attn@V flush + usb evictions) lands
            # right after this call's first tile is already in flight
            if prelude and j == 0:
                while prelude:
                    prelude.pop(0)()
            # attn@V trails by ~2 stages, emitted at per-kc granularity so
            # the in-order PE stream is never gated and bursts stay small.
            avq.append(attnv(2 * j, pt, 0))
            avq.append(attnv(2 * j + 1, pt, 1))
            while len(avq) > 4:
                avq.pop(0)()
            if trickle and (j % 3 == 2):
                trickle.pop(0)()
        for work in trickle:
            work()
        usb = [usbp.tile([65, 512], F32, tag="usb", name=f"usb{pair}_{qw}_{h2}")
               for h2 in range(2)]

        def _tail():
            for work in avq:
                work()
            nc.scalar.copy(usb[0], U[0])
            nc.vector.tensor_copy(usb[1], U[1])

        def norm_one(h2, qlo=0, qwidth=512):
            def go():
                u = usb[h2]
                rc = rcpool.tile([65, 512], BF16, tag="rc",
                                 name=f"rc{pair}_{qw}_{h2}_{qlo}")
                with nc.allow_low_precision(reason="1/D broadcast in bf16"):
                    nc.vector.reciprocal(rc[64:65, qlo:qlo + qwidth],
                                         u[64:65, qlo:qlo + qwidth])
                ut = psp.tile([128, 1024], F32, tag="ps",
                              name=f"nm{pair}_{qw}_{h2}_{qlo}")
                rb = ut[0:64, 0:512]
                nc.tensor.matmul(rb[:, 0:qwidth], lhsT=ones_sb[64:65, :],
                                 rhs=rc[64:65, qlo:qlo + qwidth],
                                 start=True, stop=True)
                un = rcpool.tile([64, 512], BF16, tag="un",
                                 name=f"un{pair}_{qw}_{h2}_{qlo}")
                nc.vector.scalar_tensor_tensor(
                    un[:, 0:qwidth], u[0:64, qlo:qlo + qwidth], 1.0,
                    rb[:, 0:qwidth], ALU.mult, ALU.mult)
                cx = ut[:, 512:1024]
                nc.tensor.matmul(cx[h2 * 64:(h2 + 1) * 64, 0:qwidth],
                                 lhsT=wv_sb, rhs=un[:, 0:qwidth],
                                 start=True, stop=True)
                nc.vector.tensor_copy(
                    concatT[pair][h2 * 64:(h2 + 1) * 64,
                                  qw * 512 + qlo:qw * 512 + qlo + qwidth],
                    cx[h2 * 64:(h2 + 1) * 64, 0:qwidth])
            return go

        if last:
            _tail()
            return [norm_one(0, 0, 256), norm_one(1, 0, 256),
                    norm_one(0, 256, 256), norm_one(1, 256, 256)], None
        return [norm_one(0), norm_one(1)], _tail

    osb_t = {}

    def outproj(qts):
        def one(qt):
            def go():
                ut = psp.tile([128, 1024], F32, tag="ps", name=f"op{qt}")
                op = ut[:, 0:512]
                for pc in range(4):
                    nc.tensor.matmul(op,
                                     lhsT=concatT[pc][:, qt * 128:(qt + 1) * 128],
                                     rhs=wo_sb[:, pc * E:(pc + 1) * E],
                                     start=(pc == 0), stop=(pc == 3))
                osb = ospool.tile([128, E], F32, tag="osb", name=f"osb{qt}")
                osb_t[qt] = osb
                nc.vector.scalar_tensor_tensor(osb, op, 1.0, bo_sb,
                                               ALU.mult, ALU.add)
                nc.sync.dma_start(out[qt * 128:(qt + 1) * 128, :], osb)
            return go
        return [one(qt) for qt in qts]

    # ---- emission schedule (qw-major call order) ----
    qproj_unit(0, 0)()
    qproj_unit(0, 1)()
    load_bulk()
    n00, t00 = attn(0, 0, trickle=[qproj_unit(1, 0), qproj_unit(1, 1),
                                   qproj_unit(2, 0)])
    n10, t10 = attn(1, 0, trickle=n00 + [qproj_unit(2, 1)], prelude=[t00])
    n20, t20 = attn(2, 0, trickle=n10 + [qproj_unit(3, 0)], prelude=[t10])
    n30, t30 = attn(3, 0, trickle=n20 + [qproj_unit(3, 1)], prelude=[t20])
    n01, t01 = attn(0, 1, trickle=n30, prelude=[t30])
    n11, t11 = attn(1, 1, trickle=n01, prelude=[t01])
    n21, t21 = attn(2, 1, trickle=n11, prelude=[t11])
    # outproj for q rows 0..511 only needs the qw0 norms (all done by now)
    op03 = outproj(range(4))
    n31, _ = attn(3, 1, trickle=n21 + op03, prelude=[t21], last=True)
    op47 = outproj(range(4, 8))
    n31[0]()
    n31[1]()
    op47[0]()
    op47[1]()
    n31[2]()
    n31[3]()
    op47[2]()
    op47[3]()

    ctx.close()


def _prep_inputs(key, query, value, mask, Wq, Wk, Wv, Wo, bo):
    bf16 = ml_dtypes.bfloat16
    key = np.asarray(key, np.float32)
    query = np.asarray(query, np.float32)
    value = np.asarray(value, np.float32)
    mask = np.asarray(mask)
    Wq = np.asarray(Wq, np.float32)
    Wk = np.asarray(Wk, np.float32)

    M = (Wq.T @ Wk).astype(np.float32)
    wm = np.concatenate([M, M], axis=0)  # [128, 64]

    common = {
        "wmd": wm.astype(bf16),
        "wvT": np.ascontiguousarray(np.asarray(Wv, np.float32).T).astype(bf16),
        "wod": np.ascontiguousarray(
            np.asarray(Wo, np.float32).T.reshape(4, 128, E)
            .transpose(1, 0, 2).reshape(128, 4 * E)).astype(bf16),
        "bo_bd": np.ascontiguousarray(
            np.broadcast_to(np.asarray(bo, np.float32), (128, E))),
    }

    per_b = {}
    for b in range(B):
        v4 = value[b].reshape(KC, 128, H, DH).transpose(1, 0, 2, 3)
        vb = np.empty((128, KC, H, 65), np.float32)
        vb[:, :, :, :DH] = v4
        vb[:, :, :, DH] = 1.0
        per_b[b] = {
            "xkT": np.ascontiguousarray(key[b].T).astype(bf16),
            "valpd": np.ascontiguousarray(vb.reshape(128, KC * VBLK)).astype(bf16),
            "qT": query[b].T,
        }

    mT = (mask[0, 0] != 0).T  # [k, q]
    slot_order = sorted(ASSIST_SLOTS)

    in_maps = []
    for c in range(N_CORES):
        b, qs = c // 4, (c % 4) * QLEN
        mk = mT[:, qs:qs + QLEN].reshape(KC, 128, QLEN).transpose(1, 0, 2)
        m16 = mk.astype(np.float32).reshape(128, KC * QLEN)
        m32 = np.empty((128, len(slot_order) * 512), np.float32)
        for i, s in enumerate(slot_order):
            kc, qw = s // 2, s % 2
            m32[:, i * 512:(i + 1) * 512] = \
                mk[:, kc, qw * 512:(qw + 1) * 512] * SCH_SCALE
        in_maps.append({
            "xqT": np.ascontiguousarray(
                per_b[b]["qT"][:, qs:qs + QLEN]).astype(bf16),
            "xkT": per_b[b]["xkT"],
            "valpd": per_b[b]["valpd"],
            "m16d": np.ascontiguousarray(m16).astype(bf16),
            "m32d": np.ascontiguousarray(m32),
            **common,
        })
    return in_maps


def get_module():
    if "nc" not in _CACHE:
        _CACHE["nc"] = _build_module()
    return _CACHE["nc"]


def kernel(key, query, value, mask, Wq, Wk, Wv, Wo, bo, **_):
    nc = get_module()
    in_maps = _prep_inputs(key, query, value, mask, Wq, Wk, Wv, Wo, bo)
    res = bass_utils.run_bass_kernel_spmd(
        nc, in_maps, core_ids=list(range(N_CORES)))
    full = np.empty((B, S, E), np.float32)
    for c in range(N_CORES):
        b, qs = c // 4, (c % 4) * QLEN
        full[b, qs:qs + QLEN, :] = res.results[c]["out"]
    return full


# revision 35
# speedup vs baseline: 1.0113x; 1.0113x over previous
"""Trainium2 Bass kernel for 8-head MultiHeadAttention (B=2, S=4096, E=512).

Sharding: 8 cores = 2 batches x 4 query-row chunks of 1024; each core computes
all 8 heads for its (batch, q-range), so there are no collectives.

Structure (v3):
- scores = (Wq xq)^T (Wk xk) = (M^T xq)^T xk with M = Wq^T Wk folded on host:
  the key side needs NO on-device projection (raw key columns are the
  stationary operand), and only the small q-side projection runs on the PE.
- QK^T and attn@V run in bf16 (fp8 was tried and measured: the heavy-tailed
  softmax here has n_eff ~ 2..10, so fp8's quantization noise does not
  average out and blows the 2e-2 gate).
- exp runs on ACT for ~90% of tiles; ~10% of (kc,qw) slots are offloaded to
  the DVE via a Schraudolph bit-trick: int16(round(s*128*log2(e)/8 + 16248.67))
  IS the bf16 bit pattern of ~exp(s/8), computed by one scalar_tensor_tensor
  ((ps + 703.921) * m32) where m32 is {23.0831, 0} so the mask rides along and
  masked lanes become +0.0. The 16248.67 (vs 16256) bias zero-means the
  mantissa-interpolation sawtooth so softmax cancels it; measured L2 8e-3.
- masks for ACT tiles are multiplicative bf16 (DVE at 2x, some tiles on Pool).
- V is unprojected (+ ones column for the denominator); Wv is applied after
  normalization on the small [64, q] normalized context. The output bias bo
  is added by a K=1 ones-row matmul into the outproj PSUM.
"""
import sys
for _p in ('/root/.axon_site/_ro/trn_rl_repo', '/opt/trn_rl_repo'):
    if _p not in sys.path:
        sys.path.append(_p)

import numpy as np
import ml_dtypes

import concourse.bass as bass
import concourse.tile as tile
from concourse import bacc, mybir
from concourse import bass_utils

F32 = mybir.dt.float32
BF16 = mybir.dt.bfloat16
I16 = mybir.dt.int16
AF = mybir.ActivationFunctionType
ALU = mybir.AluOpType

N_CORES = 8
B, S, E, H, DH = 2, 4096, 512, 8, 64
QLEN = S // 4           # 1024 q rows per core
KC = S // 128           # 32 k chunks
VBLK = H * 65           # 520: per-kc block in valp

# Schraudolph (int16 -> bf16 bits): y16 = s*128*log2(e)/8 + (16256 - 7.33)
SCH_SCALE = 23.083120654223414        # 128*log2(e)/8
SCH_BIAS = 16248.67 / SCH_SCALE       # 703.9209...: (ps + bias)*m32

# (kc,qw) slot classes: s = 2*kc + qw
ASSIST_SLOTS = frozenset({18, 26, 42, 58, 17, 33, 49, 21})      # DVE exp (8)
POOL_SLOTS = frozenset(s for s in range(64)
                       if s % 12 == 2 and s not in ASSIST_SLOTS)  # Pool mask

_CACHE = {}


def _build_module():
    nc = bacc.Bacc("TRN2", target_bir_lowering=False, debug=False,
                   enable_asserts=True, num_devices=N_CORES)

    xqT = nc.dram_tensor("xqT", [E, QLEN], BF16, kind="ExternalInput").ap()
    xkT = nc.dram_tensor("xkT", [E, S], BF16, kind="ExternalInput").ap()
    valpd = nc.dram_tensor("valpd", [128, KC * VBLK], BF16, kind="ExternalInput").ap()
    m16d = nc.dram_tensor("m16d", [128, KC * QLEN], BF16, kind="ExternalInput").ap()
    m32d = nc.dram_tensor("m32d", [128, len(ASSIST_SLOTS) * 512], F32,
                          kind="ExternalInput").ap()
    wmd = nc.dram_tensor("wmd", [128, DH], BF16, kind="ExternalInput").ap()
    wvT = nc.dram_tensor("wvT", [DH, DH], BF16, kind="ExternalInput").ap()
    wod = nc.dram_tensor("wod", [128, 4 * E], BF16, kind="ExternalInput").ap()
    bo_bd = nc.dram_tensor("bo_bd", [128, E], F32, kind="ExternalInput").ap()
    out = nc.dram_tensor("out", [QLEN, E], F32, kind="ExternalOutput").ap()

    with tile.TileContext(nc) as tc:
        _emit(tc, nc, xqT, xkT, valpd, m16d, m32d, wmd, wvT, wod, bo_bd, out)

    nc.compile()
    return nc


def _emit(tc, nc, xqT, xkT, valpd, m16d, m32d, wmd, wvT, wod, bo_bd, out):
    from contextlib import ExitStack
    ctx = ExitStack()
    const = ctx.enter_context(tc.tile_pool(name="const", bufs=1))
    qpool = ctx.enter_context(tc.tile_pool(name="qp", bufs=1))
    xqpool = ctx.enter_context(tc.tile_pool(name="xq", bufs=1))
    ptpool = ctx.enter_context(tc.tile_pool(name="pt", bufs=3))
    usbp = ctx.enter_context(tc.tile_pool(name="usb", bufs=4))
    rcpool = ctx.enter_context(tc.tile_pool(name="rc", bufs=4))
    ospool = ctx.enter_context(tc.tile_pool(name="osb", bufs=2))
    psp = ctx.enter_context(tc.tile_pool(name="psp", bufs=3, space="PSUM"))
    uacc = ctx.enter_context(tc.tile_pool(name="uacc", bufs=2, space="PSUM"))

    # ---- small constants (critical path first) ----
    wm_sb = const.tile([128, DH], BF16, tag="wm")   # M at rows 0:64 and 64:128
    nc.sync.dma_start(wm_sb, wmd)
    ones_sb = const.tile([65, DH], BF16, tag="ones")
    nc.vector.memset(ones_sb, 1.0)

    xq_sb = []   # per pair: [128, QLEN] bf16, head-even rows 0:64, odd 64:128
    for pair in range(4):
        xq_sb.append(xqpool.tile([128, QLEN], BF16, tag=f"xq{pair}",
                                 name=f"xq{pair}"))

    def load_xq(pair):
        nc.sync.dma_start(xq_sb[pair], xqT[(2 * pair) * DH:(2 * pair + 2) * DH, :])

    load_xq(0)

    xkp = []     # per pair: [128, S] bf16, head-even rows 0:64, odd 64:128
    for pair in range(4):
        xkp.append(const.tile([128, S], BF16, tag=f"xk{pair}", name=f"xk{pair}"))

    def load_xk(pair):
        nc.sync.dma_start(xkp[pair], xkT[(2 * pair) * DH:(2 * pair + 2) * DH, :])

    nc.sync.dma_start(xkp[0][:, 0:1024], xkT[0:128, 0:1024])

    valp = const.tile([128, KC * VBLK], BF16, tag="valp")
    m16 = const.tile([128, KC * QLEN], BF16, tag="m16")
    m32 = const.tile([128, len(ASSIST_SLOTS) * 512], F32, tag="m32")
    wv_sb = const.tile([DH, DH], BF16, tag="wv")
    wo_sb = const.tile([128, 4 * E], BF16, tag="wo")
    bo_sb = const.tile([128, E], F32, tag="bo")

    def m16_chunk(qw, kclo, kchi):
        """Strided (qw-half, kc-range) slice of m16 / m16d for need-ordered DMA."""
        n = kchi - kclo
        dst = bass.AP(tensor=m16.tensor,
                      offset=m16.offset + kclo * QLEN + qw * 512,
                      ap=[m16.ap[0], [QLEN, n], [1, 512]])
        srcbase = m16d[:, 0:]
        srcd = bass.AP(tensor=srcbase.tensor,
                       offset=srcbase.offset + kclo * QLEN + qw * 512,
                       ap=[srcbase.ap[0], [QLEN, n], [1, 512]])
        return dst, srcd

    # Startup ladder: call 0 is DMA-paced, everything in strict need-order.
    nc.sync.dma_start(*m16_chunk(0, 0, 4))
    nc.sync.dma_start(valp[:, 0:4 * VBLK], valpd[:, 0:4 * VBLK])
    nc.gpsimd.dma_start(xq_sb[1], xqT[2 * DH:4 * DH, :])
    nc.gpsimd.dma_start(xq_sb[2], xqT[4 * DH:6 * DH, :])
    nc.gpsimd.dma_start(xq_sb[3], xqT[6 * DH:8 * DH, :])
    nc.gpsimd.dma_start(m32, m32d)

    def load_bulk():
        nc.sync.dma_start(*m16_chunk(0, 4, 10))
        nc.sync.dma_start(valp[:, 4 * VBLK:10 * VBLK], valpd[:, 4 * VBLK:10 * VBLK])
        nc.sync.dma_start(xkp[0][:, 1024:4096], xkT[0:128, 1024:4096])
        nc.sync.dma_start(*m16_chunk(0, 10, 18))
        nc.sync.dma_start(valp[:, 10 * VBLK:18 * VBLK], valpd[:, 10 * VBLK:18 * VBLK])
        nc.sync.dma_start(*m16_chunk(0, 18, 26))
        nc.sync.dma_start(valp[:, 18 * VBLK:26 * VBLK], valpd[:, 18 * VBLK:26 * VBLK])
        nc.sync.dma_start(*m16_chunk(0, 26, 32))
        nc.sync.dma_start(valp[:, 26 * VBLK:32 * VBLK], valpd[:, 26 * VBLK:32 * VBLK])
        load_xk(1)
        nc.sync.dma_start(*m16_chunk(1, 0, 16))
        nc.sync.dma_start(*m16_chunk(1, 16, 32))
        nc.gpsimd.dma_start(wv_sb, wvT)
        nc.gpsimd.dma_start(xkp[2], xkT[4 * DH:6 * DH, :])
        nc.gpsimd.dma_start(xkp[3], xkT[6 * DH:8 * DH, :])
        nc.gpsimd.dma_start(wo_sb, wod)
        nc.gpsimd.dma_start(bo_sb, bo_bd)

    qp = [qpool.tile([128, QLEN], BF16, tag=f"qp{pair}", name=f"qp{pair}")
          for pair in range(4)]

    # ---- q-side projection: qp[pair] = bf16(M^T xq) ----
    def qproj_unit(pair, qc):
        def go():
            ut = psp.tile([128, 1024], F32, tag="ps", name=f"qp{pair}_{qc}")
            for h2 in range(2):
                nc.tensor.matmul(
                    ut[h2 * 64:(h2 + 1) * 64, 0:512],
                    lhsT=wm_sb[h2 * 64:(h2 + 1) * 64, :],
                    rhs=xq_sb[pair][h2 * 64:(h2 + 1) * 64,
                                    qc * 512:(qc + 1) * 512],
                    start=True, stop=True)
            nc.vector.tensor_copy(qp[pair][:, qc * 512:(qc + 1) * 512],
                                  ut[:, 0:512])
        return go

    def m16_ap(kc, qw):
        t = m16[:, kc * QLEN + qw * 512:]
        return bass.AP(tensor=t.tensor, offset=t.offset,
                       ap=[t.ap[0], [0, 2], [1, 512]])

    def m32_ap(slot_idx):
        t = m32[:, slot_idx * 512:]
        return bass.AP(tensor=t.tensor, offset=t.offset,
                       ap=[t.ap[0], [0, 2], [1, 512]])

    _slot_order = sorted(ASSIST_SLOTS)

    concatT = [const.tile([128, QLEN], BF16, tag=f"ct{pair}", name=f"ct{pair}")
               for pair in range(4)]

    def attn(pair, qw, trickle=(), prelude=(), last=False):
        trickle = list(trickle)
        prelude = list(prelude)
        U = [uacc.tile([65, 512], F32, tag="u", name=f"U{pair}_{qw}_{h2}")
             for h2 in range(2)]

        def attnv(kc, pt, half):
            def go():
                for h2 in range(2):
                    nc.tensor.matmul(
                        U[h2],
                        lhsT=valp[:, kc * VBLK + (2 * pair + h2) * 65:
                                  kc * VBLK + (2 * pair + h2) * 65 + 65],
                        rhs=pt[:, half * 1024 + h2 * 512:
                               half * 1024 + (h2 + 1) * 512],
                        start=(kc == 0), stop=(kc == KC - 1),
                        skip_group_check=True)
            return go

        avq = []
        for j in range(KC // 2):
            pt = ptpool.tile([128, 2048], BF16, tag="pt",
                             name=f"pt{pair}_{qw}_{j}")
            pti16 = pt.bitcast(I16)
            for half in range(2):
                kc = 2 * j + half
                slot = 2 * kc + qw
                ps = psp.tile([128, 1024], F32, tag="ps",
                              name=f"ps{pair}_{qw}_{kc}")
                for h2 in range(2):
                    nc.tensor.matmul(
                        ps[:, h2 * 512:(h2 + 1) * 512],
                        lhsT=xkp[pair][h2 * 64:(h2 + 1) * 64,
                                       kc * 128:(kc + 1) * 128],
                        rhs=qp[pair][h2 * 64:(h2 + 1) * 64,
                                     qw * 512:(qw + 1) * 512],
                        start=True, stop=True)
                dst = pt[:, half * 1024:(half + 1) * 1024]
                if slot in ASSIST_SLOTS:
                    dsti = pti16[:, half * 1024:(half + 1) * 1024]
                    dv = bass.AP(tensor=dsti.tensor, offset=dsti.offset,
                                 ap=[dsti.ap[0], [512, 2], [1, 512]])
                    pv = bass.AP(tensor=ps.tensor, offset=ps.offset,
                                 ap=[ps.ap[0], [512, 2], [1, 512]])
                    nc.vector.scalar_tensor_tensor(
                        dv, pv, SCH_BIAS, m32_ap(_slot_order.index(slot)),
                        ALU.add, ALU.mult)
                else:
                    nc.scalar.activation(dst, ps, AF.Exp, bias=0.0, scale=0.125)
                    dv = bass.AP(tensor=dst.tensor, offset=dst.offset,
                                 ap=[dst.ap[0], [512, 2], [1, 512]])
                    eng = nc.gpsimd if slot in POOL_SLOTS else nc.vector
                    eng.tensor_mul(dv, dv, m16_ap(kc, qw))
            # the previous call's tail (attn@V flush + usb evictions) lands
            # right after this call's first tile is already in flight
            if prelude and j == 0:
                while prelude:
                    prelude.pop(0)()
            # attn@V trails by ~2 stages, emitted at per-kc granularity so
            # the in-order PE stream is never gated and bursts stay small.
            avq.append(attnv(2 * j, pt, 0))
            avq.append(attnv(2 * j + 1, pt, 1))
            while len(avq) > 4:
                avq.pop(0)()
            if trickle and (j % 3 == 2):
                trickle.pop(0)()
        for work in trickle:
            work()
        usb = [usbp.tile([65, 512], F32, tag="usb", name=f"usb{pair}_{qw}_{h2}")
               for h2 in range(2)]

        def _tail():
            for work in avq:
                work()
            nc.scalar.copy(usb[0], U[0])
            nc.vector.tensor_copy(usb[1], U[1])

        def norm_one(h2, qlo=0, qwidth=512):
            def go():
                u = usb[h2]
                rc = rcpool.tile([65, 512], BF16, tag="rc",
                                 name=f"rc{pair}_{qw}_{h2}_{qlo}")
                with nc.allow_low_precision(reason="1/D broadcast in bf16"):
                    nc.vector.reciprocal(rc[64:65, qlo:qlo + qwidth],
                                         u[64:65, qlo:qlo + qwidth])
                ut = psp.tile([128, 1024], F32, tag="ps",
                              name=f"nm{pair}_{qw}_{h2}_{qlo}")
                rb = ut[0:64, 0:512]
                nc.tensor.matmul(rb[:, 0:qwidth], lhsT=ones_sb[64:65, :],
                                 rhs=rc[64:65, qlo:qlo + qwidth],
                                 start=True, stop=True)
                un = rcpool.tile([64, 512], BF16, tag="un",
                                 name=f"un{pair}_{qw}_{h2}_{qlo}")
                nc.vector.scalar_tensor_tensor(
                    un[:, 0:qwidth], u[0:64, qlo:qlo + qwidth], 1.0,
                    rb[:, 0:qwidth], ALU.mult, ALU.mult)
                cx = ut[:, 512:1024]
                nc.tensor.matmul(cx[h2 * 64:(h2 + 1) * 64, 0:qwidth],
                                 lhsT=wv_sb, rhs=un[:, 0:qwidth],
                                 start=True, stop=True)
                nc.vector.tensor_copy(
                    concatT[pair][h2 * 64:(h2 + 1) * 64,
                                  qw * 512 + qlo:qw * 512 + qlo + qwidth],
                    cx[h2 * 64:(h2 + 1) * 64, 0:qwidth])
            return go

        if last:
            _tail()
            return [norm_one(0, 0, 256), norm_one(1, 0, 256),
                    norm_one(0, 256, 256), norm_one(1, 256, 256)], None
        return [norm_one(0), norm_one(1)], _tail

    osb_t = {}

    def outproj(qts):
        def one(qt):
            def go():
                ut = psp.tile([128, 1024], F32, tag="ps", name=f"op{qt}")
                op = ut[:, 0:512]
                for pc in range(4):
                    nc.tensor.matmul(op,
                                     lhsT=concatT[pc][:, qt * 128:(qt + 1) * 128],
                                     rhs=wo_sb[:, pc * E:(pc + 1) * E],
                                     start=(pc == 0), stop=(pc == 3))
                osb = ospool.tile([128, E], F32, tag="osb", name=f"osb{qt}")
                osb_t[qt] = osb
                nc.vector.scalar_tensor_tensor(osb, op, 1.0, bo_sb,
                                               ALU.mult, ALU.add)
                nc.sync.dma_start(out[qt * 128:(qt + 1) * 128, :], osb)
            return go
        return [one(qt) for qt in qts]

    # ---- emission schedule (qw-major call order) ----
    qproj_unit(0, 0)()
    qproj_unit(0, 1)()
    load_bulk()
    n00, t00 = attn(0, 0, trickle=[qproj_unit(1, 0), qproj_unit(1, 1),
                                   qproj_unit(2, 0)])
    n10, t10 = attn(1, 0, trickle=n00 + [qproj_unit(2, 1)], prelude=[t00])
    n20, t20 = attn(2, 0, trickle=n10 + [qproj_unit(3, 0)], prelude=[t10])
    n30, t30 = attn(3, 0, trickle=n20 + [qproj_unit(3, 1)], prelude=[t20])
    n01, t01 = attn(0, 1, trickle=n30, prelude=[t30])
    n11, t11 = attn(1, 1, trickle=n01, prelude=[t01])
    n21, t21 = attn(2, 1, trickle=n11, prelude=[t11])
    # outproj for q rows 0..511 only needs the qw0 norms (all done by now)
    op03 = outproj(range(4))
    n31, _ = attn(3, 1, trickle=n21 + op03, prelude=[t21], last=True)
    op47 = outproj(range(4, 8))
    n31[0]()
    n31[1]()
    op47[0]()
    op47[1]()
    n31[2]()
    n31[3]()
    op47[2]()
    op47[3]()

    ctx.close()


def _prep_inputs(key, query, value, mask, Wq, Wk, Wv, Wo, bo):
    bf16 = ml_dtypes.bfloat16
    key = np.asarray(key, np.float32)
    query = np.asarray(query, np.float32)
    value = np.asarray(value, np.float32)
    mask = np.asarray(mask)
    Wq = np.asarray(Wq, np.float32)
    Wk = np.asarray(Wk, np.float32)

    M = (Wq.T @ Wk).astype(np.float32)
    wm = np.concatenate([M, M], axis=0)  # [128, 64]

    common = {
        "wmd": wm.astype(bf16),
        "wvT": np.ascontiguousarray(np.asarray(Wv, np.float32).T).astype(bf16),
        "wod": np.ascontiguousarray(
            np.asarray(Wo, np.float32).T.reshape(4, 128, E)
            .transpose(1, 0, 2).reshape(128, 4 * E)).astype(bf16),
        "bo_bd": np.ascontiguousarray(
            np.broadcast_to(np.asarray(bo, np.float32), (128, E))),
    }

    per_b = {}
    for b in range(B):
        v4 = value[b].reshape(KC, 128, H, DH).transpose(1, 0, 2, 3)
        vb = np.empty((128, KC, H, 65), np.float32)
        vb[:, :, :, :DH] = v4
        vb[:, :, :, DH] = 1.0
        per_b[b] = {
            "xkT": np.ascontiguousarray(key[b].T).astype(bf16),
            "valpd": np.ascontiguousarray(vb.reshape(128, KC * VBLK)).astype(bf16),
            "qT": query[b].T,
        }

    mT = (mask[0, 0] != 0).T  # [k, q]
    slot_order = sorted(ASSIST_SLOTS)

    in_maps = []
    for c in range(N_CORES):
        b, qs = c // 4, (c % 4) * QLEN
        mk = mT[:, qs:qs + QLEN].reshape(KC, 128, QLEN).transpose(1, 0, 2)
        m16 = mk.astype(np.float32).reshape(128, KC * QLEN)
        m32 = np.empty((128, len(slot_order) * 512), np.float32)
        for i, s in enumerate(slot_order):
            kc, qw = s // 2, s % 2
            m32[:, i * 512:(i + 1) * 512] = \
                mk[:, kc, qw * 512:(qw + 1) * 512] * SCH_SCALE
        in_maps.append({
            "xqT": np.ascontiguousarray(
                per_b[b]["qT"][:, qs:qs + QLEN]).astype(bf16),
            "xkT": per_b[b]["xkT"],
            "valpd": per_b[b]["valpd"],
            "m16d": np.ascontiguousarray(m16).astype(bf16),
            "m32d": np.ascontiguousarray(m32),
            **common,
        })
    return in_maps


def get_module():
    if "nc" not in _CACHE:
        _CACHE["nc"] = _build_module()
    return _CACHE["nc"]


def kernel(key, query, value, mask, Wq, Wk, Wv, Wo, bo, **_):
    nc = get_module()
    in_maps = _prep_inputs(key, query, value, mask, Wq, Wk, Wv, Wo, bo)
    res = bass_utils.run_bass_kernel_spmd(
        nc, in_maps, core_ids=list(range(N_CORES)))
    full = np.empty((B, S, E), np.float32)
    for c in range(N_CORES):
        b, qs = c // 4, (c % 4) * QLEN
        full[b, qs:qs + QLEN, :] = res.results[c]["out"]
    return full


# revision 36
# speedup vs baseline: 1.0353x; 1.0237x over previous
"""Trainium2 Bass kernel for 8-head MultiHeadAttention (B=2, S=4096, E=512).

Sharding: 8 cores = 2 batches x 4 query-row chunks of 1024; each core computes
all 8 heads for its (batch, q-range), so there are no collectives.

Structure (v3):
- scores = (Wq xq)^T (Wk xk) = (M^T xq)^T xk with M = Wq^T Wk folded on host:
  the key side needs NO on-device projection (raw key columns are the
  stationary operand), and only the small q-side projection runs on the PE.
- QK^T and attn@V run in bf16 (fp8 was tried and measured: the heavy-tailed
  softmax here has n_eff ~ 2..10, so fp8's quantization noise does not
  average out and blows the 2e-2 gate).
- exp runs on ACT for ~90% of tiles; ~10% of (kc,qw) slots are offloaded to
  the DVE via a Schraudolph bit-trick: int16(round(s*128*log2(e)/8 + 16248.67))
  IS the bf16 bit pattern of ~exp(s/8), computed by one scalar_tensor_tensor
  ((ps + 703.921) * m32) where m32 is {23.0831, 0} so the mask rides along and
  masked lanes become +0.0. The 16248.67 (vs 16256) bias zero-means the
  mantissa-interpolation sawtooth so softmax cancels it; measured L2 8e-3.
- masks for ACT tiles are multiplicative bf16 (DVE at 2x, some tiles on Pool).
- V is unprojected (+ ones column for the denominator); Wv is applied after
  normalization on the small [64, q] normalized context. The output bias bo
  is added by a K=1 ones-row matmul into the outproj PSUM.
"""
import sys
for _p in ('/root/.axon_site/_ro/trn_rl_repo', '/opt/trn_rl_repo'):
    if _p not in sys.path:
        sys.path.append(_p)

import numpy as np
import ml_dtypes

import concourse.bass as bass
import concourse.tile as tile
from concourse import bacc, mybir
from concourse import bass_utils

F32 = mybir.dt.float32
BF16 = mybir.dt.bfloat16
I16 = mybir.dt.int16
AF = mybir.ActivationFunctionType
ALU = mybir.AluOpType

N_CORES = 8
B, S, E, H, DH = 2, 4096, 512, 8, 64
QLEN = S // 4           # 1024 q rows per core
KC = S // 128           # 32 k chunks
VBLK = H * 65           # 520: per-kc block in valp

# Schraudolph (int16 -> bf16 bits): y16 = s*128*log2(e)/8 + (16256 - 7.33)
SCH_SCALE = 23.083120654223414        # 128*log2(e)/8
SCH_BIAS = 16248.67 / SCH_SCALE       # 703.9209...: (ps + bias)*m32

# (kc,qw) slot classes: s = 2*kc + qw
ASSIST_SLOTS = frozenset({18, 26, 42, 58, 17, 33, 49, 21})      # DVE exp (8)
POOL_SLOTS = frozenset(s for s in range(64)
                       if s % 24 == 2 and s not in ASSIST_SLOTS)  # Pool mask

_CACHE = {}


def _build_module():
    nc = bacc.Bacc("TRN2", target_bir_lowering=False, debug=False,
                   enable_asserts=True, num_devices=N_CORES)

    xqT = nc.dram_tensor("xqT", [E, QLEN], BF16, kind="ExternalInput").ap()
    xkT = nc.dram_tensor("xkT", [E, S], BF16, kind="ExternalInput").ap()
    valpd = nc.dram_tensor("valpd", [128, KC * VBLK], BF16, kind="ExternalInput").ap()
    m16d = nc.dram_tensor("m16d", [128, KC * QLEN], BF16, kind="ExternalInput").ap()
    m32d = nc.dram_tensor("m32d", [128, len(ASSIST_SLOTS) * 512], F32,
                          kind="ExternalInput").ap()
    wmd = nc.dram_tensor("wmd", [128, DH], BF16, kind="ExternalInput").ap()
    wvT = nc.dram_tensor("wvT", [DH, DH], BF16, kind="ExternalInput").ap()
    wod = nc.dram_tensor("wod", [128, 4 * E], BF16, kind="ExternalInput").ap()
    bo_bd = nc.dram_tensor("bo_bd", [128, E], F32, kind="ExternalInput").ap()
    out = nc.dram_tensor("out", [QLEN, E], F32, kind="ExternalOutput").ap()

    with tile.TileContext(nc) as tc:
        _emit(tc, nc, xqT, xkT, valpd, m16d, m32d, wmd, wvT, wod, bo_bd, out)

    nc.compile()
    return nc


def _emit(tc, nc, xqT, xkT, valpd, m16d, m32d, wmd, wvT, wod, bo_bd, out):
    from contextlib import ExitStack
    ctx = ExitStack()
    const = ctx.enter_context(tc.tile_pool(name="const", bufs=1))
    qpool = ctx.enter_context(tc.tile_pool(name="qp", bufs=1))
    xqpool = ctx.enter_context(tc.tile_pool(name="xq", bufs=1))
    ptpool = ctx.enter_context(tc.tile_pool(name="pt", bufs=3))
    usbp = ctx.enter_context(tc.tile_pool(name="usb", bufs=4))
    rcpool = ctx.enter_context(tc.tile_pool(name="rc", bufs=4))
    ospool = ctx.enter_context(tc.tile_pool(name="osb", bufs=2))
    psp = ctx.enter_context(tc.tile_pool(name="psp", bufs=3, space="PSUM"))
    uacc = ctx.enter_context(tc.tile_pool(name="uacc", bufs=2, space="PSUM"))

    # ---- small constants (critical path first) ----
    wm_sb = const.tile([128, DH], BF16, tag="wm")   # M at rows 0:64 and 64:128
    nc.sync.dma_start(wm_sb, wmd)
    ones_sb = const.tile([65, DH], BF16, tag="ones")
    nc.vector.memset(ones_sb, 1.0)

    xq_sb = []   # per pair: [128, QLEN] bf16, head-even rows 0:64, odd 64:128
    for pair in range(4):
        xq_sb.append(xqpool.tile([128, QLEN], BF16, tag=f"xq{pair}",
                                 name=f"xq{pair}"))

    def load_xq(pair):
        nc.sync.dma_start(xq_sb[pair], xqT[(2 * pair) * DH:(2 * pair + 2) * DH, :])

    load_xq(0)

    xkp = []     # per pair: [128, S] bf16, head-even rows 0:64, odd 64:128
    for pair in range(4):
        xkp.append(const.tile([128, S], BF16, tag=f"xk{pair}", name=f"xk{pair}"))

    def load_xk(pair):
        nc.sync.dma_start(xkp[pair], xkT[(2 * pair) * DH:(2 * pair + 2) * DH, :])

    nc.sync.dma_start(xkp[0][:, 0:1024], xkT[0:128, 0:1024])

    valp = const.tile([128, KC * VBLK], BF16, tag="valp")
    m16 = const.tile([128, KC * QLEN], BF16, tag="m16")
    m32 = const.tile([128, len(ASSIST_SLOTS) * 512], F32, tag="m32")
    wv_sb = const.tile([DH, DH], BF16, tag="wv")
    wo_sb = const.tile([128, 4 * E], BF16, tag="wo")
    bo_sb = const.tile([128, E], F32, tag="bo")

    def m16_chunk(qw, kclo, kchi):
        """Strided (qw-half, kc-range) slice of m16 / m16d for need-ordered DMA."""
        n = kchi - kclo
        dst = bass.AP(tensor=m16.tensor,
                      offset=m16.offset + kclo * QLEN + qw * 512,
                      ap=[m16.ap[0], [QLEN, n], [1, 512]])
        srcbase = m16d[:, 0:]
        srcd = bass.AP(tensor=srcbase.tensor,
                       offset=srcbase.offset + kclo * QLEN + qw * 512,
                       ap=[srcbase.ap[0], [QLEN, n], [1, 512]])
        return dst, srcd

    # Startup ladder: call 0 is DMA-paced, everything in strict need-order.
    nc.sync.dma_start(*m16_chunk(0, 0, 4))
    nc.sync.dma_start(valp[:, 0:4 * VBLK], valpd[:, 0:4 * VBLK])
    nc.gpsimd.dma_start(xq_sb[1], xqT[2 * DH:4 * DH, :])
    nc.gpsimd.dma_start(xq_sb[2], xqT[4 * DH:6 * DH, :])
    nc.gpsimd.dma_start(xq_sb[3], xqT[6 * DH:8 * DH, :])
    nc.gpsimd.dma_start(m32, m32d)

    def load_bulk():
        nc.sync.dma_start(*m16_chunk(0, 4, 10))
        nc.sync.dma_start(valp[:, 4 * VBLK:10 * VBLK], valpd[:, 4 * VBLK:10 * VBLK])
        nc.sync.dma_start(xkp[0][:, 1024:4096], xkT[0:128, 1024:4096])
        nc.sync.dma_start(*m16_chunk(0, 10, 18))
        nc.sync.dma_start(valp[:, 10 * VBLK:18 * VBLK], valpd[:, 10 * VBLK:18 * VBLK])
        nc.sync.dma_start(*m16_chunk(0, 18, 26))
        nc.sync.dma_start(valp[:, 18 * VBLK:26 * VBLK], valpd[:, 18 * VBLK:26 * VBLK])
        nc.sync.dma_start(*m16_chunk(0, 26, 32))
        nc.sync.dma_start(valp[:, 26 * VBLK:32 * VBLK], valpd[:, 26 * VBLK:32 * VBLK])
        load_xk(1)
        nc.sync.dma_start(*m16_chunk(1, 0, 16))
        nc.sync.dma_start(*m16_chunk(1, 16, 32))
        nc.gpsimd.dma_start(wv_sb, wvT)
        nc.gpsimd.dma_start(xkp[2], xkT[4 * DH:6 * DH, :])
        nc.gpsimd.dma_start(xkp[3], xkT[6 * DH:8 * DH, :])
        nc.gpsimd.dma_start(wo_sb, wod)
        nc.gpsimd.dma_start(bo_sb, bo_bd)

    qp = [qpool.tile([128, QLEN], BF16, tag=f"qp{pair}", name=f"qp{pair}")
          for pair in range(4)]

    # ---- q-side projection: qp[pair] = bf16(M^T xq) ----
    def qproj_unit(pair, qc):
        def go():
            ut = psp.tile([128, 1024], F32, tag="ps", name=f"qp{pair}_{qc}")
            for h2 in range(2):
                nc.tensor.matmul(
                    ut[h2 * 64:(h2 + 1) * 64, 0:512],
                    lhsT=wm_sb[h2 * 64:(h2 + 1) * 64, :],
                    rhs=xq_sb[pair][h2 * 64:(h2 + 1) * 64,
                                    qc * 512:(qc + 1) * 512],
                    start=True, stop=True)
            nc.vector.tensor_copy(qp[pair][:, qc * 512:(qc + 1) * 512],
                                  ut[:, 0:512])
        return go

    def m16_ap(kc, qw):
        t = m16[:, kc * QLEN + qw * 512:]
        return bass.AP(tensor=t.tensor, offset=t.offset,
                       ap=[t.ap[0], [0, 2], [1, 512]])

    def m32_ap(slot_idx):
        t = m32[:, slot_idx * 512:]
        return bass.AP(tensor=t.tensor, offset=t.offset,
                       ap=[t.ap[0], [0, 2], [1, 512]])

    _slot_order = sorted(ASSIST_SLOTS)

    concatT = [const.tile([128, QLEN], BF16, tag=f"ct{pair}", name=f"ct{pair}")
               for pair in range(4)]

    def attn(pair, qw, trickle=(), prelude=(), last=False):
        trickle = list(trickle)
        prelude = list(prelude)
        U = [uacc.tile([65, 512], F32, tag="u", name=f"U{pair}_{qw}_{h2}")
             for h2 in range(2)]

        def attnv(kc, pt, half):
            def go():
                for h2 in range(2):
                    nc.tensor.matmul(
                        U[h2],
                        lhsT=valp[:, kc * VBLK + (2 * pair + h2) * 65:
                                  kc * VBLK + (2 * pair + h2) * 65 + 65],
                        rhs=pt[:, half * 1024 + h2 * 512:
                               half * 1024 + (h2 + 1) * 512],
                        start=(kc == 0), stop=(kc == KC - 1),
                        skip_group_check=True)
            return go

        avq = []
        for j in range(KC // 2):
            pt = ptpool.tile([128, 2048], BF16, tag="pt",
                             name=f"pt{pair}_{qw}_{j}")
            pti16 = pt.bitcast(I16)
            for half in range(2):
                kc = 2 * j + half
                slot = 2 * kc + qw
                ps = psp.tile([128, 1024], F32, tag="ps",
                              name=f"ps{pair}_{qw}_{kc}")
                for h2 in range(2):
                    nc.tensor.matmul(
                        ps[:, h2 * 512:(h2 + 1) * 512],
                        lhsT=xkp[pair][h2 * 64:(h2 + 1) * 64,
                                       kc * 128:(kc + 1) * 128],
                        rhs=qp[pair][h2 * 64:(h2 + 1) * 64,
                                     qw * 512:(qw + 1) * 512],
                        start=True, stop=True)
                dst = pt[:, half * 1024:(half + 1) * 1024]
                if slot in ASSIST_SLOTS:
                    dsti = pti16[:, half * 1024:(half + 1) * 1024]
                    dv = bass.AP(tensor=dsti.tensor, offset=dsti.offset,
                                 ap=[dsti.ap[0], [512, 2], [1, 512]])
                    pv = bass.AP(tensor=ps.tensor, offset=ps.offset,
                                 ap=[ps.ap[0], [512, 2], [1, 512]])
                    nc.vector.scalar_tensor_tensor(
                        dv, pv, SCH_BIAS, m32_ap(_slot_order.index(slot)),
                        ALU.add, ALU.mult)
                else:
                    nc.scalar.activation(dst, ps, AF.Exp, bias=0.0, scale=0.125)
                    dv = bass.AP(tensor=dst.tensor, offset=dst.offset,
                                 ap=[dst.ap[0], [512, 2], [1, 512]])
                    eng = nc.gpsimd if slot in POOL_SLOTS else nc.vector
                    eng.tensor_mul(dv, dv, m16_ap(kc, qw))
            # the previous call's tail (attn@V flush + usb evictions) lands
            # right after this call's first tile is already in flight
            if prelude and j == 0:
                while prelude:
                    prelude.pop(0)()
            # attn@V trails by ~2 stages, emitted at per-kc granularity so
            # the in-order PE stream is never gated and bursts stay small.
            avq.append(attnv(2 * j, pt, 0))
            avq.append(attnv(2 * j + 1, pt, 1))
            while len(avq) > 4:
                avq.pop(0)()
            if trickle and (j % 3 == 2):
                trickle.pop(0)()
        for work in trickle:
            work()
        usb = [usbp.tile([65, 512], F32, tag="usb", name=f"usb{pair}_{qw}_{h2}")
               for h2 in range(2)]

        def _tail():
            for work in avq:
                work()
            nc.scalar.copy(usb[0], U[0])
            nc.vector.tensor_copy(usb[1], U[1])

        def norm_one(h2, qlo=0, qwidth=512):
            def go():
                u = usb[h2]
                rc = rcpool.tile([65, 512], BF16, tag="rc",
                                 name=f"rc{pair}_{qw}_{h2}_{qlo}")
                with nc.allow_low_precision(reason="1/D broadcast in bf16"):
                    nc.vector.reciprocal(rc[64:65, qlo:qlo + qwidth],
                                         u[64:65, qlo:qlo + qwidth])
                ut = psp.tile([128, 1024], F32, tag="ps",
                              name=f"nm{pair}_{qw}_{h2}_{qlo}")
                rb = ut[0:64, 0:512]
                nc.tensor.matmul(rb[:, 0:qwidth], lhsT=ones_sb[64:65, :],
                                 rhs=rc[64:65, qlo:qlo + qwidth],
                                 start=True, stop=True)
                un = rcpool.tile([64, 512], BF16, tag="un",
                                 name=f"un{pair}_{qw}_{h2}_{qlo}")
                nc.vector.scalar_tensor_tensor(
                    un[:, 0:qwidth], u[0:64, qlo:qlo + qwidth], 1.0,
                    rb[:, 0:qwidth], ALU.mult, ALU.mult)
                cx = ut[:, 512:1024]
                nc.tensor.matmul(cx[h2 * 64:(h2 + 1) * 64, 0:qwidth],
                                 lhsT=wv_sb, rhs=un[:, 0:qwidth],
                                 start=True, stop=True)
                nc.vector.tensor_copy(
                    concatT[pair][h2 * 64:(h2 + 1) * 64,
                                  qw * 512 + qlo:qw * 512 + qlo + qwidth],
                    cx[h2 * 64:(h2 + 1) * 64, 0:qwidth])
            return go

        if last:
            _tail()
            return [norm_one(0, 0, 256), norm_one(1, 0, 256),
                    norm_one(0, 256, 256), norm_one(1, 256, 256)], None
        return [norm_one(0), norm_one(1)], _tail

    osb_t = {}

    def outproj(qts):
        def one(qt):
            def go():
                ut = psp.tile([128, 1024], F32, tag="ps", name=f"op{qt}")
                op = ut[:, 0:512]
                for pc in range(4):
                    nc.tensor.matmul(op,
                                     lhsT=concatT[pc][:, qt * 128:(qt + 1) * 128],
                                     rhs=wo_sb[:, pc * E:(pc + 1) * E],
                                     start=(pc == 0), stop=(pc == 3))
                osb = ospool.tile([128, E], F32, tag="osb", name=f"osb{qt}")
                osb_t[qt] = osb
                nc.vector.scalar_tensor_tensor(osb, op, 1.0, bo_sb,
                                               ALU.mult, ALU.add)
                nc.sync.dma_start(out[qt * 128:(qt + 1) * 128, :], osb)
            return go
        return [one(qt) for qt in qts]

    # ---- emission schedule (qw-major call order) ----
    qproj_unit(0, 0)()
    qproj_unit(0, 1)()
    load_bulk()
    n00, t00 = attn(0, 0, trickle=[qproj_unit(1, 0), qproj_unit(1, 1),
                                   qproj_unit(2, 0)])
    n10, t10 = attn(1, 0, trickle=n00 + [qproj_unit(2, 1)], prelude=[t00])
    n20, t20 = attn(2, 0, trickle=n10 + [qproj_unit(3, 0)], prelude=[t10])
    n30, t30 = attn(3, 0, trickle=n20 + [qproj_unit(3, 1)], prelude=[t20])
    n01, t01 = attn(0, 1, trickle=n30, prelude=[t30])
    n11, t11 = attn(1, 1, trickle=n01, prelude=[t01])
    n21, t21 = attn(2, 1, trickle=n11, prelude=[t11])
    # outproj for q rows 0..511 only needs the qw0 norms (all done by now)
    op03 = outproj(range(4))
    n31, _ = attn(3, 1, trickle=n21 + op03, prelude=[t21], last=True)
    op47 = outproj(range(4, 8))
    n31[0]()
    n31[1]()
    op47[0]()
    op47[1]()
    n31[2]()
    n31[3]()
    op47[2]()
    op47[3]()

    ctx.close()


def _prep_inputs(key, query, value, mask, Wq, Wk, Wv, Wo, bo):
    bf16 = ml_dtypes.bfloat16
    key = np.asarray(key, np.float32)
    query = np.asarray(query, np.float32)
    value = np.asarray(value, np.float32)
    mask = np.asarray(mask)
    Wq = np.asarray(Wq, np.float32)
    Wk = np.asarray(Wk, np.float32)

    M = (Wq.T @ Wk).astype(np.float32)
    wm = np.concatenate([M, M], axis=0)  # [128, 64]

    common = {
        "wmd": wm.astype(bf16),
        "wvT": np.ascontiguousarray(np.asarray(Wv, np.float32).T).astype(bf16),
        "wod": np.ascontiguousarray(
            np.asarray(Wo, np.float32).T.reshape(4, 128, E)
            .transpose(1, 0, 2).reshape(128, 4 * E)).astype(bf16),
        "bo_bd": np.ascontiguousarray(
            np.broadcast_to(np.asarray(bo, np.float32), (128, E))),
    }

    per_b = {}
    for b in range(B):
        v4 = value[b].reshape(KC, 128, H, DH).transpose(1, 0, 2, 3)
        vb = np.empty((128, KC, H, 65), np.float32)
        vb[:, :, :, :DH] = v4
        vb[:, :, :, DH] = 1.0
        per_b[b] = {
            "xkT": np.ascontiguousarray(key[b].T).astype(bf16),
            "valpd": np.ascontiguousarray(vb.reshape(128, KC * VBLK)).astype(bf16),
            "qT": query[b].T,
        }

    mT = (mask[0, 0] != 0).T  # [k, q]
    slot_order = sorted(ASSIST_SLOTS)

    in_maps = []
    for c in range(N_CORES):
        b, qs = c // 4, (c % 4) * QLEN
        mk = mT[:, qs:qs + QLEN].reshape(KC, 128, QLEN).transpose(1, 0, 2)
        m16 = mk.astype(np.float32).reshape(128, KC * QLEN)
        m32 = np.empty((128, len(slot_order) * 512), np.float32)
        for i, s in enumerate(slot_order):
            kc, qw = s // 2, s % 2
            m32[:, i * 512:(i + 1) * 512] = \
                mk[:, kc, qw * 512:(qw + 1) * 512] * SCH_SCALE
        in_maps.append({
            "xqT": np.ascontiguousarray(
                per_b[b]["qT"][:, qs:qs + QLEN]).astype(bf16),
            "xkT": per_b[b]["xkT"],
            "valpd": per_b[b]["valpd"],
            "m16d": np.ascontiguousarray(m16).astype(bf16),
            "m32d": np.ascontiguousarray(m32),
            **common,
        })
    return in_maps


def get_module():
    if "nc" not in _CACHE:
        _CACHE["nc"] = _build_module()
    return _CACHE["nc"]


def kernel(key, query, value, mask, Wq, Wk, Wv, Wo, bo, **_):
    nc = get_module()
    in_maps = _prep_inputs(key, query, value, mask, Wq, Wk, Wv, Wo, bo)
    res = bass_utils.run_bass_kernel_spmd(
        nc, in_maps, core_ids=list(range(N_CORES)))
    full = np.empty((B, S, E), np.float32)
    for c in range(N_CORES):
        b, qs = c // 4, (c % 4) * QLEN
        full[b, qs:qs + QLEN, :] = res.results[c]["out"]
    return full


# revision 37
# speedup vs baseline: 1.0573x; 1.0213x over previous
"""Trainium2 Bass kernel for 8-head MultiHeadAttention (B=2, S=4096, E=512).

Sharding: 8 cores = 2 batches x 4 query-row chunks of 1024; each core computes
all 8 heads for its (batch, q-range), so there are no collectives.

Structure (v3):
- scores = (Wq xq)^T (Wk xk) = (M^T xq)^T xk with M = Wq^T Wk folded on host:
  the key side needs NO on-device projection (raw key columns are the
  stationary operand), and only the small q-side projection runs on the PE.
- QK^T and attn@V run in bf16 (fp8 was tried and measured: the heavy-tailed
  softmax here has n_eff ~ 2..10, so fp8's quantization noise does not
  average out and blows the 2e-2 gate).
- exp runs on ACT for ~90% of tiles; ~10% of (kc,qw) slots are offloaded to
  the DVE via a Schraudolph bit-trick: int16(round(s*128*log2(e)/8 + 16248.67))
  IS the bf16 bit pattern of ~exp(s/8), computed by one scalar_tensor_tensor
  ((ps + 703.921) * m32) where m32 is {23.0831, 0} so the mask rides along and
  masked lanes become +0.0. The 16248.67 (vs 16256) bias zero-means the
  mantissa-interpolation sawtooth so softmax cancels it; measured L2 8e-3.
- masks for ACT tiles are multiplicative bf16 (DVE at 2x, some tiles on Pool).
- V is unprojected (+ ones column for the denominator); Wv is applied after
  normalization on the small [64, q] normalized context. The output bias bo
  is added by a K=1 ones-row matmul into the outproj PSUM.
"""
import sys
for _p in ('/root/.axon_site/_ro/trn_rl_repo', '/opt/trn_rl_repo'):
    if _p not in sys.path:
        sys.path.append(_p)

import numpy as np
import ml_dtypes

import concourse.bass as bass
import concourse.tile as tile
from concourse import bacc, mybir
from concourse import bass_utils

F32 = mybir.dt.float32
BF16 = mybir.dt.bfloat16
I16 = mybir.dt.int16
AF = mybir.ActivationFunctionType
ALU = mybir.AluOpType

N_CORES = 8
B, S, E, H, DH = 2, 4096, 512, 8, 64
QLEN = S // 4           # 1024 q rows per core
KC = S // 128           # 32 k chunks
VBLK = H * 65           # 520: per-kc block in valp

# Schraudolph (int16 -> bf16 bits): y16 = s*128*log2(e)/8 + (16256 - 7.33)
SCH_SCALE = 23.083120654223414        # 128*log2(e)/8
SCH_BIAS = 16248.67 / SCH_SCALE       # 703.9209...: (ps + bias)*m32

# (kc,qw) slot classes: s = 2*kc + qw
ASSIST_SLOTS = frozenset({18, 26, 42, 58, 17, 33, 49, 21})      # DVE exp (8)
POOL_SLOTS = frozenset()  # Pool masks measured net-negative: stall attn@V

_CACHE = {}


def _build_module():
    nc = bacc.Bacc("TRN2", target_bir_lowering=False, debug=False,
                   enable_asserts=True, num_devices=N_CORES)

    xqT = nc.dram_tensor("xqT", [E, QLEN], BF16, kind="ExternalInput").ap()
    xkT = nc.dram_tensor("xkT", [E, S], BF16, kind="ExternalInput").ap()
    valpd = nc.dram_tensor("valpd", [128, KC * VBLK], BF16, kind="ExternalInput").ap()
    m16d = nc.dram_tensor("m16d", [128, KC * QLEN], BF16, kind="ExternalInput").ap()
    m32d = nc.dram_tensor("m32d", [128, len(ASSIST_SLOTS) * 512], F32,
                          kind="ExternalInput").ap()
    wmd = nc.dram_tensor("wmd", [128, DH], BF16, kind="ExternalInput").ap()
    wvT = nc.dram_tensor("wvT", [DH, DH], BF16, kind="ExternalInput").ap()
    wod = nc.dram_tensor("wod", [128, 4 * E], BF16, kind="ExternalInput").ap()
    bo_bd = nc.dram_tensor("bo_bd", [128, E], F32, kind="ExternalInput").ap()
    out = nc.dram_tensor("out", [QLEN, E], F32, kind="ExternalOutput").ap()

    with tile.TileContext(nc) as tc:
        _emit(tc, nc, xqT, xkT, valpd, m16d, m32d, wmd, wvT, wod, bo_bd, out)

    nc.compile()
    return nc


def _emit(tc, nc, xqT, xkT, valpd, m16d, m32d, wmd, wvT, wod, bo_bd, out):
    from contextlib import ExitStack
    ctx = ExitStack()
    const = ctx.enter_context(tc.tile_pool(name="const", bufs=1))
    qpool = ctx.enter_context(tc.tile_pool(name="qp", bufs=1))
    xqpool = ctx.enter_context(tc.tile_pool(name="xq", bufs=1))
    ptpool = ctx.enter_context(tc.tile_pool(name="pt", bufs=3))
    usbp = ctx.enter_context(tc.tile_pool(name="usb", bufs=4))
    rcpool = ctx.enter_context(tc.tile_pool(name="rc", bufs=4))
    ospool = ctx.enter_context(tc.tile_pool(name="osb", bufs=2))
    psp = ctx.enter_context(tc.tile_pool(name="psp", bufs=3, space="PSUM"))
    uacc = ctx.enter_context(tc.tile_pool(name="uacc", bufs=2, space="PSUM"))

    # ---- small constants (critical path first) ----
    wm_sb = const.tile([128, DH], BF16, tag="wm")   # M at rows 0:64 and 64:128
    nc.sync.dma_start(wm_sb, wmd)
    ones_sb = const.tile([65, DH], BF16, tag="ones")
    nc.vector.memset(ones_sb, 1.0)

    xq_sb = []   # per pair: [128, QLEN] bf16, head-even rows 0:64, odd 64:128
    for pair in range(4):
        xq_sb.append(xqpool.tile([128, QLEN], BF16, tag=f"xq{pair}",
                                 name=f"xq{pair}"))

    def load_xq(pair):
        nc.sync.dma_start(xq_sb[pair], xqT[(2 * pair) * DH:(2 * pair + 2) * DH, :])

    load_xq(0)

    xkp = []     # per pair: [128, S] bf16, head-even rows 0:64, odd 64:128
    for pair in range(4):
        xkp.append(const.tile([128, S], BF16, tag=f"xk{pair}", name=f"xk{pair}"))

    def load_xk(pair):
        nc.sync.dma_start(xkp[pair], xkT[(2 * pair) * DH:(2 * pair + 2) * DH, :])

    nc.sync.dma_start(xkp[0][:, 0:1024], xkT[0:128, 0:1024])

    valp = const.tile([128, KC * VBLK], BF16, tag="valp")
    m16 = const.tile([128, KC * QLEN], BF16, tag="m16")
    m32 = const.tile([128, len(ASSIST_SLOTS) * 512], F32, tag="m32")
    wv_sb = const.tile([DH, DH], BF16, tag="wv")
    wo_sb = const.tile([128, 4 * E], BF16, tag="wo")
    bo_sb = const.tile([128, E], F32, tag="bo")

    def m16_chunk(qw, kclo, kchi):
        """Strided (qw-half, kc-range) slice of m16 / m16d for need-ordered DMA."""
        n = kchi - kclo
        dst = bass.AP(tensor=m16.tensor,
                      offset=m16.offset + kclo * QLEN + qw * 512,
                      ap=[m16.ap[0], [QLEN, n], [1, 512]])
        srcbase = m16d[:, 0:]
        srcd = bass.AP(tensor=srcbase.tensor,
                       offset=srcbase.offset + kclo * QLEN + qw * 512,
                       ap=[srcbase.ap[0], [QLEN, n], [1, 512]])
        return dst, srcd

    # Startup ladder: call 0 is DMA-paced, everything in strict need-order.
    nc.sync.dma_start(*m16_chunk(0, 0, 4))
    nc.sync.dma_start(valp[:, 0:4 * VBLK], valpd[:, 0:4 * VBLK])
    nc.gpsimd.dma_start(xq_sb[1], xqT[2 * DH:4 * DH, :])
    nc.gpsimd.dma_start(xq_sb[2], xqT[4 * DH:6 * DH, :])
    nc.gpsimd.dma_start(xq_sb[3], xqT[6 * DH:8 * DH, :])
    nc.gpsimd.dma_start(m32, m32d)

    def load_bulk():
        nc.sync.dma_start(*m16_chunk(0, 4, 10))
        nc.sync.dma_start(valp[:, 4 * VBLK:10 * VBLK], valpd[:, 4 * VBLK:10 * VBLK])
        nc.sync.dma_start(xkp[0][:, 1024:4096], xkT[0:128, 1024:4096])
        nc.sync.dma_start(*m16_chunk(0, 10, 18))
        nc.sync.dma_start(valp[:, 10 * VBLK:18 * VBLK], valpd[:, 10 * VBLK:18 * VBLK])
        nc.sync.dma_start(*m16_chunk(0, 18, 26))
        nc.sync.dma_start(valp[:, 18 * VBLK:26 * VBLK], valpd[:, 18 * VBLK:26 * VBLK])
        nc.sync.dma_start(*m16_chunk(0, 26, 32))
        nc.sync.dma_start(valp[:, 26 * VBLK:32 * VBLK], valpd[:, 26 * VBLK:32 * VBLK])
        load_xk(1)
        nc.sync.dma_start(*m16_chunk(1, 0, 16))
        nc.sync.dma_start(*m16_chunk(1, 16, 32))
        nc.gpsimd.dma_start(wv_sb, wvT)
        nc.gpsimd.dma_start(xkp[2], xkT[4 * DH:6 * DH, :])
        nc.gpsimd.dma_start(xkp[3], xkT[6 * DH:8 * DH, :])
        nc.gpsimd.dma_start(wo_sb, wod)
        nc.gpsimd.dma_start(bo_sb, bo_bd)

    qp = [qpool.tile([128, QLEN], BF16, tag=f"qp{pair}", name=f"qp{pair}")
          for pair in range(4)]

    # ---- q-side projection: qp[pair] = bf16(M^T xq) ----
    def qproj_unit(pair, qc):
        def go():
            ut = psp.tile([128, 1024], F32, tag="ps", name=f"qp{pair}_{qc}")
            for h2 in range(2):
                nc.tensor.matmul(
                    ut[h2 * 64:(h2 + 1) * 64, 0:512],
                    lhsT=wm_sb[h2 * 64:(h2 + 1) * 64, :],
                    rhs=xq_sb[pair][h2 * 64:(h2 + 1) * 64,
                                    qc * 512:(qc + 1) * 512],
                    start=True, stop=True)
            nc.vector.tensor_copy(qp[pair][:, qc * 512:(qc + 1) * 512],
                                  ut[:, 0:512])
        return go

    def m16_ap(kc, qw):
        t = m16[:, kc * QLEN + qw * 512:]
        return bass.AP(tensor=t.tensor, offset=t.offset,
                       ap=[t.ap[0], [0, 2], [1, 512]])

    def m32_ap(slot_idx):
        t = m32[:, slot_idx * 512:]
        return bass.AP(tensor=t.tensor, offset=t.offset,
                       ap=[t.ap[0], [0, 2], [1, 512]])

    _slot_order = sorted(ASSIST_SLOTS)

    concatT = [const.tile([128, QLEN], BF16, tag=f"ct{pair}", name=f"ct{pair}")
               for pair in range(4)]

    def attn(pair, qw, trickle=(), prelude=(), last=False):
        trickle = list(trickle)
        prelude = list(prelude)
        U = [uacc.tile([65, 512], F32, tag="u", name=f"U{pair}_{qw}_{h2}")
             for h2 in range(2)]

        def attnv(kc, pt, half):
            def go():
                for h2 in range(2):
                    nc.tensor.matmul(
                        U[h2],
                        lhsT=valp[:, kc * VBLK + (2 * pair + h2) * 65:
                                  kc * VBLK + (2 * pair + h2) * 65 + 65],
                        rhs=pt[:, half * 1024 + h2 * 512:
                               half * 1024 + (h2 + 1) * 512],
                        start=(kc == 0), stop=(kc == KC - 1),
                        skip_group_check=True)
            return go

        avq = []
        for j in range(KC // 2):
            pt = ptpool.tile([128, 2048], BF16, tag="pt",
                             name=f"pt{pair}_{qw}_{j}")
            pti16 = pt.bitcast(I16)
            for half in range(2):
                kc = 2 * j + half
                slot = 2 * kc + qw
                ps = psp.tile([128, 1024], F32, tag="ps",
                              name=f"ps{pair}_{qw}_{kc}")
                for h2 in range(2):
                    nc.tensor.matmul(
                        ps[:, h2 * 512:(h2 + 1) * 512],
                        lhsT=xkp[pair][h2 * 64:(h2 + 1) * 64,
                                       kc * 128:(kc + 1) * 128],
                        rhs=qp[pair][h2 * 64:(h2 + 1) * 64,
                                     qw * 512:(qw + 1) * 512],
                        start=True, stop=True)
                dst = pt[:, half * 1024:(half + 1) * 1024]
                if slot in ASSIST_SLOTS:
                    dsti = pti16[:, half * 1024:(half + 1) * 1024]
                    dv = bass.AP(tensor=dsti.tensor, offset=dsti.offset,
                                 ap=[dsti.ap[0], [512, 2], [1, 512]])
                    pv = bass.AP(tensor=ps.tensor, offset=ps.offset,
                                 ap=[ps.ap[0], [512, 2], [1, 512]])
                    nc.vector.scalar_tensor_tensor(
                        dv, pv, SCH_BIAS, m32_ap(_slot_order.index(slot)),
                        ALU.add, ALU.mult)
                else:
                    nc.scalar.activation(dst, ps, AF.Exp, bias=0.0, scale=0.125)
                    dv = bass.AP(tensor=dst.tensor, offset=dst.offset,
                                 ap=[dst.ap[0], [512, 2], [1, 512]])
                    eng = nc.gpsimd if slot in POOL_SLOTS else nc.vector
                    eng.tensor_mul(dv, dv, m16_ap(kc, qw))
            # the previous call's tail (attn@V flush + usb evictions) lands
            # right after this call's first tile is already in flight
            if prelude and j == 0:
                while prelude:
                    prelude.pop(0)()
            # attn@V trails by ~2 stages, emitted at per-kc granularity so
            # the in-order PE stream is never gated and bursts stay small.
            avq.append(attnv(2 * j, pt, 0))
            avq.append(attnv(2 * j + 1, pt, 1))
            while len(avq) > 4:
                avq.pop(0)()
            if trickle and (j % 3 == 2):
                trickle.pop(0)()
        for work in trickle:
            work()
        usb = [usbp.tile([65, 512], F32, tag="usb", name=f"usb{pair}_{qw}_{h2}")
               for h2 in range(2)]

        def _tail():
            for work in avq:
                work()
            nc.scalar.copy(usb[0], U[0])
            nc.vector.tensor_copy(usb[1], U[1])

        def norm_one(h2, qlo=0, qwidth=512):
            def go():
                u = usb[h2]
                rc = rcpool.tile([65, 512], BF16, tag="rc",
                                 name=f"rc{pair}_{qw}_{h2}_{qlo}")
                with nc.allow_low_precision(reason="1/D broadcast in bf16"):
                    nc.vector.reciprocal(rc[64:65, qlo:qlo + qwidth],
                                         u[64:65, qlo:qlo + qwidth])
                ut = psp.tile([128, 1024], F32, tag="ps",
                              name=f"nm{pair}_{qw}_{h2}_{qlo}")
                rb = ut[0:64, 0:512]
                nc.tensor.matmul(rb[:, 0:qwidth], lhsT=ones_sb[64:65, :],
                                 rhs=rc[64:65, qlo:qlo + qwidth],
                                 start=True, stop=True)
                un = rcpool.tile([64, 512], BF16, tag="un",
                                 name=f"un{pair}_{qw}_{h2}_{qlo}")
                nc.vector.scalar_tensor_tensor(
                    un[:, 0:qwidth], u[0:64, qlo:qlo + qwidth], 1.0,
                    rb[:, 0:qwidth], ALU.mult, ALU.mult)
                cx = ut[:, 512:1024]
                nc.tensor.matmul(cx[h2 * 64:(h2 + 1) * 64, 0:qwidth],
                                 lhsT=wv_sb, rhs=un[:, 0:qwidth],
                                 start=True, stop=True)
                nc.vector.tensor_copy(
                    concatT[pair][h2 * 64:(h2 + 1) * 64,
                                  qw * 512 + qlo:qw * 512 + qlo + qwidth],
                    cx[h2 * 64:(h2 + 1) * 64, 0:qwidth])
            return go

        if last:
            _tail()
            return [norm_one(0, 0, 256), norm_one(1, 0, 256),
                    norm_one(0, 256, 256), norm_one(1, 256, 256)], None
        return [norm_one(0), norm_one(1)], _tail

    osb_t = {}

    def outproj(qts):
        def one(qt):
            def go():
                ut = psp.tile([128, 1024], F32, tag="ps", name=f"op{qt}")
                op = ut[:, 0:512]
                for pc in range(4):
                    nc.tensor.matmul(op,
                                     lhsT=concatT[pc][:, qt * 128:(qt + 1) * 128],
                                     rhs=wo_sb[:, pc * E:(pc + 1) * E],
                                     start=(pc == 0), stop=(pc == 3))
                osb = ospool.tile([128, E], F32, tag="osb", name=f"osb{qt}")
                osb_t[qt] = osb
                nc.vector.scalar_tensor_tensor(osb, op, 1.0, bo_sb,
                                               ALU.mult, ALU.add)
                nc.sync.dma_start(out[qt * 128:(qt + 1) * 128, :], osb)
            return go
        return [one(qt) for qt in qts]

    # ---- emission schedule (qw-major call order) ----
    qproj_unit(0, 0)()
    qproj_unit(0, 1)()
    load_bulk()
    n00, t00 = attn(0, 0, trickle=[qproj_unit(1, 0), qproj_unit(1, 1),
                                   qproj_unit(2, 0)])
    n10, t10 = attn(1, 0, trickle=n00 + [qproj_unit(2, 1)], prelude=[t00])
    n20, t20 = attn(2, 0, trickle=n10 + [qproj_unit(3, 0)], prelude=[t10])
    n30, t30 = attn(3, 0, trickle=n20 + [qproj_unit(3, 1)], prelude=[t20])
    n01, t01 = attn(0, 1, trickle=n30, prelude=[t30])
    n11, t11 = attn(1, 1, trickle=n01, prelude=[t01])
    n21, t21 = attn(2, 1, trickle=n11, prelude=[t11])
    # outproj for q rows 0..511 only needs the qw0 norms (all done by now)
    op03 = outproj(range(4))
    n31, _ = attn(3, 1, trickle=n21 + op03, prelude=[t21], last=True)
    op47 = outproj(range(4, 8))
    n31[0]()
    n31[1]()
    op47[0]()
    op47[1]()
    n31[2]()
    n31[3]()
    op47[2]()
    op47[3]()

    ctx.close()


def _prep_inputs(key, query, value, mask, Wq, Wk, Wv, Wo, bo):
    bf16 = ml_dtypes.bfloat16
    key = np.asarray(key, np.float32)
    query = np.asarray(query, np.float32)
    value = np.asarray(value, np.float32)
    mask = np.asarray(mask)
    Wq = np.asarray(Wq, np.float32)
    Wk = np.asarray(Wk, np.float32)

    M = (Wq.T @ Wk).astype(np.float32)
    wm = np.concatenate([M, M], axis=0)  # [128, 64]

    common = {
        "wmd": wm.astype(bf16),
        "wvT": np.ascontiguousarray(np.asarray(Wv, np.float32).T).astype(bf16),
        "wod": np.ascontiguousarray(
            np.asarray(Wo, np.float32).T.reshape(4, 128, E)
            .transpose(1, 0, 2).reshape(128, 4 * E)).astype(bf16),
        "bo_bd": np.ascontiguousarray(
            np.broadcast_to(np.asarray(bo, np.float32), (128, E))),
    }

    per_b = {}
    for b in range(B):
        v4 = value[b].reshape(KC, 128, H, DH).transpose(1, 0, 2, 3)
        vb = np.empty((128, KC, H, 65), np.float32)
        vb[:, :, :, :DH] = v4
        vb[:, :, :, DH] = 1.0
        per_b[b] = {
            "xkT": np.ascontiguousarray(key[b].T).astype(bf16),
            "valpd": np.ascontiguousarray(vb.reshape(128, KC * VBLK)).astype(bf16),
            "qT": query[b].T,
        }

    mT = (mask[0, 0] != 0).T  # [k, q]
    slot_order = sorted(ASSIST_SLOTS)

    in_maps = []
    for c in range(N_CORES):
        b, qs = c // 4, (c % 4) * QLEN
        mk = mT[:, qs:qs + QLEN].reshape(KC, 128, QLEN).transpose(1, 0, 2)
        m16 = mk.astype(np.float32).reshape(128, KC * QLEN)
        m32 = np.empty((128, len(slot_order) * 512), np.float32)
        for i, s in enumerate(slot_order):
            kc, qw = s // 2, s % 2
            m32[:, i * 512:(i + 1) * 512] = \
                mk[:, kc, qw * 512:(qw + 1) * 512] * SCH_SCALE
        in_maps.append({
            "xqT": np.ascontiguousarray(
                per_b[b]["qT"][:, qs:qs + QLEN]).astype(bf16),
            "xkT": per_b[b]["xkT"],
            "valpd": per_b[b]["valpd"],
            "m16d": np.ascontiguousarray(m16).astype(bf16),
            "m32d": np.ascontiguousarray(m32),
            **common,
        })
    return in_maps


def get_module():
    if "nc" not in _CACHE:
        _CACHE["nc"] = _build_module()
    return _CACHE["nc"]


def kernel(key, query, value, mask, Wq, Wk, Wv, Wo, bo, **_):
    nc = get_module()
    in_maps = _prep_inputs(key, query, value, mask, Wq, Wk, Wv, Wo, bo)
    res = bass_utils.run_bass_kernel_spmd(
        nc, in_maps, core_ids=list(range(N_CORES)))
    full = np.empty((B, S, E), np.float32)
    for c in range(N_CORES):
        b, qs = c // 4, (c % 4) * QLEN
        full[b, qs:qs + QLEN, :] = res.results[c]["out"]
    return full
